# revision 59
# baseline (speedup 1.0000x reference)
"""Self-contained Trainium2 Bass kernel for the BasicAttentionBlock problem.

Full inputs in, full outputs out. 8 NeuronCores, data-parallel over
(batch element x query-half): each core computes GroupNorm + q/k/v 1x1
convs + full-key attention for its 2048 query pixels + output projection
+ residual, entirely on-chip.

Design notes (v2):
- GroupNorm is folded into the conv weights on-chip (w' = w * a per input
  channel, conv biases recomputed from the GN shift b), so the convs
  consume raw x and the stats -> first-matmul chain is short. The k-conv
  bias cancels in softmax and is dropped.
- exp(S^T) on ACT (the bottleneck: 65536 columns/core) writes fp8 pT.
- AV and the softmax denominator are fp8 DoubleRow matmuls over key-block
  pairs (0.5 cycles/row): the denominator costs 16 matmuls/block on PE
  instead of a 31-add Pool tree, and AV halves.
- 1/den (fp32r) is broadcast across partitions with a rank-1 matmul; y
  is scaled by 1/den before the projection conv so the epilogue chain is
  short (psum can only feed one input per vector op).
- PSUM: tag 'st' = 2 x [128,3,512] S^T groups (12KB), tag 'u' = 2 x 2KB
  rotating everything else (conv chunks, AV accumulators, denominators,
  1/den broadcasts, projections) in a hand-ordered emission schedule
  that keeps the 2-slot rotation free of deadlocks.
- Only native TPB opcodes are used: the extended InstISA ops (gpsimd
  accumulates, partition_broadcast, tensor_tensor_reduce) fail codegen
  in this walrus build.
"""

import numpy as np

B = 4
C = 128
H = 64
W = 64
HW = H * W          # 4096
HALF = HW // 2      # 2048 query pixels per core
NCORES = 8
GROUPS = 8
GSIZE = C // GROUPS  # 16
EPS = 1e-5
SCL = 1.0 / np.sqrt(C)   # attention logit scale
NPIX_G = GSIZE * HW      # elements per group-norm group = 65536

_CACHE = {}


def _split_excess_waits(nc, limit=1):
    """Rewrite instructions so none carries more than `limit` sync-waits.

    The walrus build in this container rejects instructions with more than
    one sync-wait command ("Too many sync wait commands"), while Tile's
    semaphore assignment freely attaches several. Excess waits are hoisted
    onto standalone InstEventSemaphore instructions placed immediately
    before the owning instruction on the same engine queue — semantically
    identical (program order on one engine), just more instructions.
    """
    import concourse.mybir as mybir

    ctr = 0
    for f in nc.m.functions:
        for bb in f.blocks:
            new = []
            changed = False
            for inst in bb.instructions:
                si = getattr(inst, "sync_info", None)
                ow = list(si.on_wait) if si is not None else []
                if len(ow) > limit:
                    # keep register-valued waits on the original instruction
                    imm = [w for w in ow if w.wait_reg is None]
                    reg = [w for w in ow if w.wait_reg is not None]
                    keep_n = max(0, limit - len(reg))
                    hoist = imm[: len(imm) - keep_n] if keep_n < len(imm) else []
                    kept = reg + imm[len(imm) - keep_n :] if keep_n else reg
                    assert len(kept) <= max(limit, len(reg))
                    for w in hoist:
                        ev = mybir.InstEventSemaphore(
                            name=f"waitsplit_{ctr}", ins=[], outs=[]
                        )
                        ctr += 1
                        ev.engine = inst.engine
                        ev.sync_info = mybir.SyncInfo(on_wait=[w], on_update=[])
                        nc.register_instruction(ev, overwrite=True)
                        new.append(ev)
                    si.on_wait = kept
                    inst.sync_info = si
                    changed = True
                new.append(inst)
            if changed:
                bb.instructions = new


def _build_bass():
    import concourse.bass as bass
    import concourse.mybir as mybir

    fp32 = mybir.dt.float32
    bf16 = mybir.dt.bfloat16
    fp8 = mybir.dt.float8e4
    AF = mybir.ActivationFunctionType
    ALU = mybir.AluOpType
    AX = mybir.AxisListType
    DR = mybir.MatmulPerfMode.DoubleRow
    from concourse.tile import TileContext as TC

    nc = bass.Bass(trn_type="TRN2")

    # ---- I/O -----------------------------------------------------------
    x_d = nc.dram_tensor("x", [C, HALF], fp32, kind="ExternalInput")
    xbf_d = nc.dram_tensor("x_bf", [C, HW], bf16, kind="ExternalInput")
    wq_d = nc.dram_tensor("wq_t", [C, C], bf16, kind="ExternalInput")
    wk_d = nc.dram_tensor("wk_t", [C, C], bf16, kind="ExternalInput")
    wv_d = nc.dram_tensor("wv_t", [C, C], bf16, kind="ExternalInput")
    wp_d = nc.dram_tensor("wp_t", [C, C], bf16, kind="ExternalInput")
    bq_d = nc.dram_tensor("bq", [C, 1], fp32, kind="ExternalInput")
    bv_d = nc.dram_tensor("bv", [C, 1], fp32, kind="ExternalInput")
    bp_d = nc.dram_tensor("bp", [C, 1], fp32, kind="ExternalInput")
    gnb_d = nc.dram_tensor("gn_b", [C, 1], fp32, kind="ExternalInput")
    gmat_d = nc.dram_tensor("gmat", [C, GROUPS], fp32, kind="ExternalInput")
    gbc_d = nc.dram_tensor("gbc", [GROUPS, C], fp32, kind="ExternalInput")
    ones2_d = nc.dram_tensor("ones2", [C, 2, 32], fp8, kind="ExternalInput")
    oner_d = nc.dram_tensor("ones_row", [1, C], mybir.dt.float32r, kind="ExternalInput")
    out_d = nc.dram_tensor("out", [C, HALF], fp32, kind="ExternalOutput")

    with TC(nc) as tc, tc.tile_pool(name="main", bufs=1) as pool, tc.tile_pool(
        name="psum", bufs=1, space="PSUM"
    ) as psum:
        # ---- ACT table prewarm (hide the exp table load) ---------------
        dum = pool.tile([1, 2], fp32, name="dum")
        nc.vector.memset(dum[:], 0.0)

        # ---- SBUF tiles -------------------------------------------------
        x_bf = pool.tile([C, HW], bf16, name="x_bf")
        x_sb = pool.tile([C, HALF], fp32, name="x_sb")
        wq_sb = pool.tile([C, C], bf16, name="wq_sb")
        wk_sb = pool.tile([C, C], bf16, name="wk_sb")
        wv_sb = pool.tile([C, C], bf16, name="wv_sb")
        wp_sb = pool.tile([C, C], bf16, name="wp_sb")
        wqs_sb = pool.tile([C, C], bf16, name="wqs_sb")
        wks_sb = pool.tile([C, C], bf16, name="wks_sb")
        wvs_sb = pool.tile([C, C], bf16, name="wvs_sb")
        bq_sb = pool.tile([C, 1], fp32, name="bq_sb")
        oner_sb = pool.tile([1, C], mybir.dt.float32r, name="oner_sb")
        bv_sb = pool.tile([C, 1], fp32, name="bv_sb")
        bp_sb = pool.tile([C, 1], fp32, name="bp_sb")
        gnb_sb = pool.tile([C, 1], fp32, name="gnb_sb")
        gmat_sb = pool.tile([C, GROUPS], fp32, name="gmat_sb")
        gbc_sb = pool.tile([GROUPS, C], fp32, name="gbc_sb")
        ones2_sb = pool.tile([C, 2, 32], fp8, name="ones2_sb")

        # ---- DMAs -------------------------------------------------------
        # SP queue: x_bf chunks 0-3 (stats-critical), then weights/consts
        # in order of first use; ACT queue: x_bf 4-5; Pool queue: x_bf
        # 6-7, then the late consts + residual x.
        for c4 in range(2):
            sl = slice(1024 * c4, 1024 * (c4 + 1))
            nc.sync.dma_start(x_bf[:, sl], xbf_d[:, sl])
        nc.scalar.dma_start(x_bf[:, 2048:3072], xbf_d[:, 2048:3072])
        nc.gpsimd.dma_start(x_bf[:, 3072:4096], xbf_d[:, 3072:4096])
        nc.sync.dma_start(wq_sb[:], wq_d[:])
        nc.sync.dma_start(wk_sb[:], wk_d[:])
        nc.sync.dma_start(gmat_sb[:], gmat_d[:])
        nc.sync.dma_start(gbc_sb[:], gbc_d[:])
        nc.sync.dma_start(gnb_sb[:], gnb_d[:])
        nc.sync.dma_start(bq_sb[:], bq_d[:])
        nc.sync.dma_start(wv_sb[:], wv_d[:])
        nc.sync.dma_start(wp_sb[:], wp_d[:])
        # prewarm exp/ln/square table while the stats DMAs stream
        nc.scalar.activation(dum[:], dum[:], AF.Exp)
        nc.sync.dma_start(oner_sb[:], oner_d[:])
        nc.sync.dma_start(bv_sb[:], bv_d[:])
        nc.sync.dma_start(bp_sb[:], bp_d[:])
        nc.sync.dma_start(ones2_sb[:], ones2_d[:])
        for c4 in range(4):
            sl = slice(512 * c4, 512 * (c4 + 1))
            nc.sync.dma_start(x_sb[:, sl], x_d[:, sl])

        # ---- GroupNorm stats (overlap the x DMA) -----------------------
        # Sums: pairwise-add tree on Pool (otherwise idle; fp32 partials).
        # Sum of squares: split between ACT (Square + accumulate over the
        # first half; shares the exp table set) and DVE (square + reduce
        # of the second half).
        sq_scr = pool.tile([C, 2048], bf16, name="sq_scr")
        sqd_scr = pool.tile([C, 2048], bf16, name="sqd_scr")
        ssb = pool.tile([C, 2], fp32, name="ssb")
        stats = pool.tile([C, 2], fp32, name="stats")
        tv = pool.tile([C, 1024], fp32, name="tv")
        tu = pool.tile([C, 1024], fp32, name="tu")
        tw = pool.tile([C, 1024], fp32, name="tw")
        nc.gpsimd.tensor_tensor(
            tv[:], x_bf[:, 2048:3072], x_bf[:, 3072:4096], ALU.add
        )
        nc.gpsimd.tensor_tensor(tu[:], x_bf[:, 0:1024], x_bf[:, 1024:2048], ALU.add)
        nc.gpsimd.tensor_tensor(tw[:], tu[:], tv[:], ALU.add)
        lvl = tw
        for width in (512, 256, 128, 64, 32):
            nxt = pool.tile([C, width], fp32, name=f"tl{width}")
            nc.gpsimd.tensor_tensor(
                nxt[:], lvl[:, :width], lvl[:, width : 2 * width], ALU.add
            )
            lvl = nxt
        nc.vector.tensor_reduce(stats[:, 0:1], lvl[:], axis=AX.X, op=ALU.add)
        nc.vector.tensor_tensor(
            sqd_scr[:], x_bf[:, 2048:4096], x_bf[:, 2048:4096], ALU.mult
        )
        nc.scalar.activation(
            sq_scr[:], x_bf[:, 0:2048], AF.Square, accum_out=ssb[:, 0:1]
        )
        nc.vector.tensor_reduce(ssb[:, 1:2], sqd_scr[:], axis=AX.X, op=ALU.add)
        nc.vector.tensor_reduce(stats[:, 1:2], ssb[:], axis=AX.X, op=ALU.add)

        eps_sb = pool.tile([GROUPS, 1], fp32, name="eps_sb")
        nc.vector.memset(eps_sb[:], EPS)

        gsum_ps = psum.tile([GROUPS, 2], fp32, name="gsum_ps", tag="u", bufs=2)
        nc.tensor.matmul(gsum_ps[:], gmat_sb[:], stats[:], start=True, stop=True)
        # mean^2 and var read the group stats straight from PSUM (the
        # scalar operand is exempt from the one-PSUM rule)
        msq = pool.tile([GROUPS, 1], fp32, name="msq")
        nc.vector.tensor_scalar(
            msq[:], gsum_ps[:, 0:1], gsum_ps[:, 0:1], None, ALU.mult
        )
        tve = pool.tile([GROUPS, 1], fp32, name="tve")
        nc.vector.tensor_tensor(tve[:], gsum_ps[:, 1:2], msq[:], ALU.subtract)

        # rsqrt(var+eps) = exp(-0.5*ln(var+eps)); eps rides the Ln bias.
        lnt = pool.tile([GROUPS, 1], fp32, name="lnt")
        nc.scalar.activation(lnt[:], tve[:], AF.Ln, bias=eps_sb[:])
        r1 = pool.tile([GROUPS, 1], fp32, name="r1")
        nc.scalar.activation(r1[:], lnt[:], AF.Exp, scale=-0.5)
        mr = pool.tile([GROUPS, 1], fp32, name="mr")
        nc.vector.tensor_scalar(mr[:], r1[:], gsum_ps[:, 0:1], None, ALU.mult)

        # a = gn_w * rsqrt (per channel), b = gn_b - mean * a
        a_ps = psum.tile([C, 1], fp32, name="a_ps", tag="u", bufs=2)
        nc.tensor.matmul(a_ps[:], gbc_sb[:], r1[:], start=True, stop=True)
        bm_ps = psum.tile([C, 1], fp32, name="bm_ps", tag="u", bufs=2)
        nc.tensor.matmul(bm_ps[:], gbc_sb[:], mr[:], start=True, stop=True)
        # fold the GN scale into the conv weights: w'[c,o] = w_t[c,o]*a[c]
        # (the per-partition scalar reads PSUM directly: walrus's one-PSUM
        # rule only covers non-scalar inputs)
        nc.vector.tensor_scalar(wqs_sb[:], wq_sb[:], a_ps[:], None, ALU.mult)
        nc.vector.tensor_scalar(wks_sb[:], wk_sb[:], a_ps[:], None, ALU.mult)
        nc.vector.tensor_scalar(wvs_sb[:], wv_sb[:], a_ps[:], None, ALU.mult)
        b_bf = pool.tile([C, 1], bf16, name="b_bf")
        nc.vector.tensor_tensor(b_bf[:], gnb_sb[:], bm_ps[:], ALU.subtract)
        # q-conv bias column (emitted before the convs so the evacs can
        # apply it as a per-partition scalar with no ordering hazard)
        bhq_ps = psum.tile([C, 1], fp32, name="bhq_ps", tag="u", bufs=2)
        nc.tensor.matmul(bhq_ps[:], wq_sb[:], b_bf[:], start=True, stop=True)
        bhq_sb = pool.tile([C, 1], fp32, name="bhq_sb")
        nc.vector.tensor_tensor(bhq_sb[:], bhq_ps[:], bq_sb[:], ALU.add)

        def emit_biases():
            """v/p conv biases from the GN shift b (emitted after the first
            q/k conv matmuls so PE serves those first):
              bhv = wv.b + bv ; bp2 = wp.bhv + bp
            (the k-conv bias is constant per query in the logits -> cancels)
            """
            bhv_ps = psum.tile([C, 1], fp32, name="bhv_ps", tag="u", bufs=2)
            nc.tensor.matmul(bhv_ps[:], wv_sb[:], b_bf[:], start=True, stop=True)
            bhv_sb = pool.tile([C, 1], fp32, name="bhv_sb")
            nc.vector.tensor_tensor(bhv_sb[:], bhv_ps[:], bv_sb[:], ALU.add)
            bhv_bf = pool.tile([C, 1], bf16, name="bhv_bf")
            nc.vector.tensor_copy(bhv_bf[:], bhv_sb[:])
            pb_ps = psum.tile([C, 1], fp32, name="pb_ps", tag="u", bufs=2)
            nc.tensor.matmul(pb_ps[:], wp_sb[:], bhv_bf[:], start=True, stop=True)
            bp2_sb = pool.tile([C, 1], fp32, name="bp2_sb")
            nc.vector.tensor_tensor(bp2_sb[:], pb_ps[:], bp_sb[:], ALU.add)
            # xb = x + bp2 (residual + folded projection bias), on Pool
            nc.gpsimd.tensor_scalar(xb[:], x_sb[:], bp2_sb[:], None, ALU.add)

        xb = pool.tile([C, HALF], fp32, name="xb")

        # ---- conv emitters ---------------------------------------------
        k_bf = pool.tile([C, HW], bf16, name="k_bf")
        q_bf = pool.tile([C, HALF], bf16, name="q_bf")
        vT_f8 = pool.tile([C, 32, C], fp8, name="vT_f8")

        def emit_k_chunk(c8):
            sl = slice(512 * c8, 512 * (c8 + 1))
            kps = psum.tile([C, 512], fp32, name=f"kps{c8}", tag="u", bufs=2)
            nc.tensor.matmul(kps[:], wks_sb[:], x_bf[:, sl], start=True, stop=True)
            if c8 == 0:
                # split so the first S^T matmuls start per key block
                nc.vector.tensor_copy(k_bf[:, 0:128], kps[:, 0:128])
                nc.vector.tensor_copy(k_bf[:, 128:384], kps[:, 128:384])
                nc.vector.tensor_copy(k_bf[:, 384:512], kps[:, 384:512])
            else:
                nc.vector.tensor_copy(k_bf[:, sl], kps[:])

        def emit_q_chunk(c4):
            sl = slice(512 * c4, 512 * (c4 + 1))
            qps = psum.tile([C, 512], fp32, name=f"qps{c4}", tag="u", bufs=2)
            nc.tensor.matmul(qps[:], wqs_sb[:], x_bf[:, sl], start=True, stop=True)
            if c4 == 0:
                # ACT is idle pre-body: overlap the q evac with the k evac
                nc.scalar.activation(q_bf[:, sl], qps[:], AF.Identity, bias=bhq_sb[:])
            else:
                nc.vector.tensor_scalar(q_bf[:, sl], qps[:], bhq_sb[:], None, ALU.add)

        def emit_v_chunk(g8):
            vps = psum.tile([C, 512], fp32, name=f"vps{g8}", tag="u", bufs=2)
            for m in range(4):
                jb = 4 * g8 + m
                nc.tensor.matmul(
                    vps[:, 128 * m : 128 * (m + 1)],
                    x_bf[:, 128 * jb : 128 * (jb + 1)],
                    wvs_sb[:],
                    start=True,
                    stop=True,
                )
            nc.vector.tensor_copy(
                vT_f8[:, 4 * g8 : 4 * (g8 + 1), :],
                vps[:].rearrange("p (m c) -> p m c", m=4),
            )

        emit_q_chunk(0)
        emit_k_chunk(0)
        emit_biases()

        # ---- attention --------------------------------------------------
        jgroups = [(3 * g, 3) for g in range(10)] + [(30, 2)]
        n_ib = HALF // 512  # 4 query blocks of 512
        pT_tiles = [None] * n_ib
        yps_tiles = [None] * n_ib
        den_tiles = [None] * n_ib
        out_sb = pool.tile([C, HALF], fp32, name="out_sb")

        conv_state = {"k": 1, "q": 1, "v": 0}
        # conv chunk emission schedule for block 0 (group -> jobs).
        blk0_jobs = {
            0: ["k", "v"], 1: ["k", "v"], 2: ["k", "v"], 3: ["k", "q", "v"],
            4: ["k", "v"], 5: ["k", "q", "v"], 6: ["k", "v"], 7: ["k", "q", "v"],
        }

        def run_conv_job(j):
            if j == "k" and conv_state["k"] < 8:
                emit_k_chunk(conv_state["k"])
                conv_state["k"] += 1
            elif j == "q" and conv_state["q"] < 4:
                emit_q_chunk(conv_state["q"])
                conv_state["q"] += 1
            elif j == "v" and conv_state["v"] < 8:
                emit_v_chunk(conv_state["v"])
                conv_state["v"] += 1

        def alloc_y(ib):
            yps_tiles[ib] = psum.tile([C, 512], fp32, name=f"yps{ib}", tag="u", bufs=2)

        def alloc_d(ib):
            # 32 identical rows: dual-fp8 ldweights needs >=32 weight cols
            den_tiles[ib] = psum.tile([32, 512], fp32, name=f"den{ib}", tag="u", bufs=2)

        def emit_av_pair(ib, p):
            nc.tensor.matmul(
                yps_tiles[ib][:],
                vT_f8[:, 2 * p : 2 * p + 2, :],
                pT_tiles[ib][:, 2 * p : 2 * p + 2, :],
                start=(p == 0),
                stop=(p == 15),
                perf_mode=DR,
            )

        def emit_den_pair(ib, p):
            nc.tensor.matmul(
                den_tiles[ib][:],
                ones2_sb[:],
                pT_tiles[ib][:, 2 * p : 2 * p + 2, :],
                start=(p == 0),
                stop=(p == 15),
                perf_mode=DR,
            )

        rbc_tiles = [None] * n_ib
        ybf_tiles = [None] * n_ib

        def emit_epi_a(ib):
            """1/den (fp32r) + y evacuation + 1/den row-broadcast matmul;
            emitted at the end of the block that ran AV(ib)/den(ib)."""
            rden = pool.tile(
                [1, 512], mybir.dt.float32r, name=f"rden{ib}", tag="rden", bufs=2
            )
            with nc.allow_low_precision(reason="fp32r broadcast of 1/den"):
                nc.vector.reciprocal(rden[:], den_tiles[ib][0:1, :])
            y_bf = pool.tile([C, 512], bf16, name=f"ybf{ib}", tag="ybf", bufs=2)
            if ib == 3:
                # ACT is idle after the last exp: do the y evac there
                nc.scalar.activation(y_bf[:], yps_tiles[ib][:], AF.Copy)
            else:
                nc.vector.tensor_copy(y_bf[:], yps_tiles[ib][:])
            ybf_tiles[ib] = y_bf
            rbc = psum.tile([C, 512], fp32, name=f"rbc{ib}", tag="u", bufs=2)
            nc.tensor.matmul(rbc[:], oner_sb[:], rden[:], start=True, stop=True)
            rbc_tiles[ib] = rbc

        def emit_epi_b(ib):
            """y/den -> proj -> + (x + bp2) -> store."""
            sl = slice(512 * ib, 512 * (ib + 1))
            if ib == 3:
                # fully halved pipeline to shorten the final drain
                y2 = pool.tile([C, 512], bf16, name=f"y2_{ib}", tag="y2", bufs=2)
                for hs in (slice(0, 256), slice(256, 512)):
                    nc.vector.tensor_tensor(
                        y2[:, hs],
                        ybf_tiles[ib][:, hs],
                        rbc_tiles[ib][:, hs],
                        ALU.mult,
                    )
                pps_h = []
                for hi, hs in enumerate((slice(0, 256), slice(256, 512))):
                    pps = psum.tile(
                        [C, 256], fp32, name=f"pps{ib}{hi}", tag="u", bufs=2
                    )
                    nc.tensor.matmul(
                        pps[:], wp_sb[:], y2[:, hs], start=True, stop=True
                    )
                    pps_h.append(pps)
                for hi, hs in enumerate((slice(0, 256), slice(256, 512))):
                    osl = slice(512 * ib + hs.start, 512 * ib + hs.stop)
                    nc.vector.tensor_tensor(
                        out_sb[:, osl], pps_h[hi][:], xb[:, osl], ALU.add
                    )
                    # the two final stores drain on separate DMA queues
                    eng = nc.sync if hi == 0 else nc.scalar
                    eng.dma_start(out_d[:, osl], out_sb[:, osl])
            else:
                y2 = pool.tile([C, 512], bf16, name=f"y2_{ib}", tag="y2", bufs=2)
                nc.vector.tensor_tensor(
                    y2[:], ybf_tiles[ib][:], rbc_tiles[ib][:], ALU.mult
                )
                pps = psum.tile([C, 512], fp32, name=f"pps{ib}", tag="u", bufs=2)
                nc.tensor.matmul(pps[:], wp_sb[:], y2[:], start=True, stop=True)
                nc.vector.tensor_tensor(out_sb[:, sl], pps[:], xb[:, sl], ALU.add)
                nc.sync.dma_start(out_d[:, sl], out_sb[:, sl])

        def emit_st_group(ib, j0, glen):
            st = psum.tile([C, glen, 512], fp32, name=f"st{ib}_{j0}", tag="st", bufs=2)
            qs = q_bf[:, 512 * ib : 512 * (ib + 1)]
            for u2 in range(glen):
                jb = j0 + u2
                nc.tensor.matmul(
                    st[:, u2, :],
                    k_bf[:, 128 * jb : 128 * (jb + 1)],
                    qs,
                    start=True,
                    stop=True,
                )
            nc.scalar.activation(
                pT_tiles[ib][:, j0 : j0 + glen, :], st[:], AF.Exp, scale=float(SCL)
            )

        # -- block 0: S^T/exp + remaining conv chunks ---------------------
        pT_tiles[0] = pool.tile([C, 32, 512], fp8, name="pT0", tag="pT", bufs=3)
        for gi, (j0, glen) in enumerate(jgroups):
            emit_st_group(0, j0, glen)
            for j in blk0_jobs.get(gi, []):
                run_conv_job(j)

        # -- block 1: + AV0/den0 spread over the block --------------------
        pT_tiles[1] = pool.tile([C, 32, 512], fp8, name="pT1", tag="pT", bufs=3)
        av_done = den_done = 0
        for gi, (j0, glen) in enumerate(jgroups):
            emit_st_group(1, j0, glen)
            if gi == 1:
                alloc_d(0)
                alloc_y(0)
            if gi >= 1:
                tgt = min(16, 2 * gi)
                while den_done < tgt:
                    emit_den_pair(0, den_done)
                    den_done += 1
                while av_done < tgt:
                    emit_av_pair(0, av_done)
                    av_done += 1
        while den_done < 16:
            emit_den_pair(0, den_done)
            den_done += 1
        while av_done < 16:
            emit_av_pair(0, av_done)
            av_done += 1
        emit_epi_a(0)

        # -- block 2: + epilogue-B(0), AV1/den1 spread --------------------
        pT_tiles[2] = pool.tile([C, 32, 512], fp8, name="pT2", tag="pT", bufs=3)
        av_done = den_done = 0
        for gi, (j0, glen) in enumerate(jgroups):
            emit_st_group(2, j0, glen)
            if gi == 0:
                emit_epi_b(0)   # praw0/tmul0 free the rbc0/pps0 slots
            if gi == 2:
                alloc_d(1)      # A slot: after tmul0 read of rbc0
                alloc_y(1)      # B slot: after praw0 read of pps0
            if gi >= 2:
                tgt = min(16, 2 * (gi - 1))
                while den_done < tgt:
                    emit_den_pair(1, den_done)
                    den_done += 1
                while av_done < tgt:
                    emit_av_pair(1, av_done)
                    av_done += 1
        while den_done < 16:
            emit_den_pair(1, den_done)
            den_done += 1
        while av_done < 16:
            emit_av_pair(1, av_done)
            av_done += 1
        emit_epi_a(1)

        # -- block 3: epi-B(1), AV2/den2 bursts, self-trailing AV3/den3 --
        pT_tiles[3] = pool.tile([C, 32, 512], fp8, name="pT3", tag="pT", bufs=3)
        av2 = den2 = 0
        av3 = den3 = 0
        for gi, (j0, glen) in enumerate(jgroups):
            emit_st_group(3, j0, glen)
            if gi == 0:
                emit_epi_b(1)
            if gi == 2:
                alloc_d(2)
                alloc_y(2)
            if 2 <= gi <= 5:
                tgt = min(16, 4 * (gi - 1))
                while den2 < tgt:
                    emit_den_pair(2, den2)
                    den2 += 1
                while av2 < tgt:
                    emit_av_pair(2, av2)
                    av2 += 1
            if gi == 5:
                emit_epi_a(2)   # recip2/y-evac2 free den2/yps2 mid-block
            if gi == 6:
                emit_epi_b(2)
            if gi >= 7:
                if den3 == 0:
                    alloc_d(3)  # A slot: after tmul2 read of rbc2
                    alloc_y(3)  # B slot: after praw2 read of pps2
                ready = min(16, (3 * gi + 1) // 2 + 1)
                while den3 < ready:
                    emit_den_pair(3, den3)
                    den3 += 1
                while av3 < ready:
                    emit_av_pair(3, av3)
                    av3 += 1
        while den3 < 16:
            emit_den_pair(3, den3)
            den3 += 1
        while av3 < 16:
            emit_av_pair(3, av3)
            av3 += 1
        emit_epi_a(3)
        emit_epi_b(3)

    _split_excess_waits(nc)
    return nc


def _get_nc():
    if "nc" not in _CACHE:
        _CACHE["nc"] = _build_bass()
    return _CACHE["nc"]


def prepare_in_maps(x, gn_w, gn_b, wq, bq, wk, bk, wv, bv, wp, bp):
    import ml_dtypes

    bf = ml_dtypes.bfloat16
    f8 = ml_dtypes.float8_e4m3
    f32 = np.float32

    x = np.asarray(x, f32)
    xf = x.reshape(B, C, HW)

    def col(v):
        return np.ascontiguousarray(np.asarray(v, f32).reshape(C, 1))

    wq_t = np.ascontiguousarray(np.asarray(wq, f32).T).astype(bf)
    wk_t = np.ascontiguousarray(np.asarray(wk, f32).T).astype(bf)
    wv_t = np.ascontiguousarray(np.asarray(wv, f32).T).astype(bf)
    wp_t = np.ascontiguousarray(np.asarray(wp, f32).T).astype(bf)

    gmat = np.zeros((C, GROUPS), f32)
    for c in range(C):
        gmat[c, c // GSIZE] = 1.0
    gbc = np.ascontiguousarray(gmat.T * np.asarray(gn_w, f32)[None, :])
    gmat = gmat * f32(1.0 / NPIX_G)

    shared = {
        "wq_t": wq_t,
        "wk_t": wk_t,
        "wv_t": wv_t,
        "wp_t": wp_t,
        "bq": col(bq),
        "bv": col(bv),
        "bp": col(bp),
        "gn_b": col(gn_b),
        "gmat": gmat,
        "gbc": gbc,
        "ones2": np.ones((C, 2, 32), f8),
        "ones_row": np.ones((1, C), f32),
    }

    in_maps = []
    for core in range(NCORES):
        b, qh = divmod(core, 2)
        if qh == 0:
            xp = np.ascontiguousarray(xf[b])
        else:
            xp = np.ascontiguousarray(
                np.concatenate([xf[b][:, HALF:], xf[b][:, :HALF]], axis=1)
            )
        in_maps.append(
            {
                "x": np.ascontiguousarray(xp[:, :HALF]),
                "x_bf": xp.astype(bf),
                **shared,
            }
        )
    return in_maps


def kernel(x, gn_w, gn_b, wq, bq, wk, bk, wv, bv, wp, bp):
    from concourse.bass_utils import run_bass_kernel_spmd

    f32 = np.float32
    in_maps = prepare_in_maps(x, gn_w, gn_b, wq, bq, wk, bk, wv, bv, wp, bp)
    nc = _get_nc()
    res = run_bass_kernel_spmd(nc, in_maps, core_ids=list(range(NCORES)))

    out = np.empty((B, C, HW), f32)
    for core in range(NCORES):
        b, qh = divmod(core, 2)
        out[b][:, HALF * qh : HALF * (qh + 1)] = res.results[core]["out"]
    return out.reshape(B, C, H, W)


# revision 60
# speedup vs baseline: 1.0011x; 1.0011x over previous
"""Self-contained Trainium2 Bass kernel for the BasicAttentionBlock problem.

Full inputs in, full outputs out. 8 NeuronCores, data-parallel over
(batch element x query-half): each core computes GroupNorm + q/k/v 1x1
convs + full-key attention for its 2048 query pixels + output projection
+ residual, entirely on-chip.

Design notes (v2):
- GroupNorm is folded into the conv weights on-chip (w' = w * a per input
  channel, conv biases recomputed from the GN shift b), so the convs
  consume raw x and the stats -> first-matmul chain is short. The k-conv
  bias cancels in softmax and is dropped.
- exp(S^T) on ACT (the bottleneck: 65536 columns/core) writes fp8 pT.
- AV and the softmax denominator are fp8 DoubleRow matmuls over key-block
  pairs (0.5 cycles/row): the denominator costs 16 matmuls/block on PE
  instead of a 31-add Pool tree, and AV halves.
- 1/den (fp32r) is broadcast across partitions with a rank-1 matmul; y
  is scaled by 1/den before the projection conv so the epilogue chain is
  short (psum can only feed one input per vector op).
- PSUM: tag 'st' = 2 x [128,3,512] S^T groups (12KB), tag 'u' = 2 x 2KB
  rotating everything else (conv chunks, AV accumulators, denominators,
  1/den broadcasts, projections) in a hand-ordered emission schedule
  that keeps the 2-slot rotation free of deadlocks.
- Only native TPB opcodes are used: the extended InstISA ops (gpsimd
  accumulates, partition_broadcast, tensor_tensor_reduce) fail codegen
  in this walrus build.
"""

import numpy as np

B = 4
C = 128
H = 64
W = 64
HW = H * W          # 4096
HALF = HW // 2      # 2048 query pixels per core
NCORES = 8
GROUPS = 8
GSIZE = C // GROUPS  # 16
EPS = 1e-5
SCL = 1.0 / np.sqrt(C)   # attention logit scale
NPIX_G = GSIZE * HW      # elements per group-norm group = 65536

_CACHE = {}


def _split_excess_waits(nc, limit=1):
    """Rewrite instructions so none carries more than `limit` sync-waits.

    The walrus build in this container rejects instructions with more than
    one sync-wait command ("Too many sync wait commands"), while Tile's
    semaphore assignment freely attaches several. Excess waits are hoisted
    onto standalone InstEventSemaphore instructions placed immediately
    before the owning instruction on the same engine queue — semantically
    identical (program order on one engine), just more instructions.
    """
    import concourse.mybir as mybir

    ctr = 0
    for f in nc.m.functions:
        for bb in f.blocks:
            new = []
            changed = False
            for inst in bb.instructions:
                si = getattr(inst, "sync_info", None)
                ow = list(si.on_wait) if si is not None else []
                if len(ow) > limit:
                    # keep register-valued waits on the original instruction
                    imm = [w for w in ow if w.wait_reg is None]
                    reg = [w for w in ow if w.wait_reg is not None]
                    keep_n = max(0, limit - len(reg))
                    hoist = imm[: len(imm) - keep_n] if keep_n < len(imm) else []
                    kept = reg + imm[len(imm) - keep_n :] if keep_n else reg
                    assert len(kept) <= max(limit, len(reg))
                    for w in hoist:
                        ev = mybir.InstEventSemaphore(
                            name=f"waitsplit_{ctr}", ins=[], outs=[]
                        )
                        ctr += 1
                        ev.engine = inst.engine
                        ev.sync_info = mybir.SyncInfo(on_wait=[w], on_update=[])
                        nc.register_instruction(ev, overwrite=True)
                        new.append(ev)
                    si.on_wait = kept
                    inst.sync_info = si
                    changed = True
                new.append(inst)
            if changed:
                bb.instructions = new


def _build_bass():
    import concourse.bass as bass
    import concourse.mybir as mybir

    fp32 = mybir.dt.float32
    bf16 = mybir.dt.bfloat16
    fp8 = mybir.dt.float8e4
    AF = mybir.ActivationFunctionType
    ALU = mybir.AluOpType
    AX = mybir.AxisListType
    DR = mybir.MatmulPerfMode.DoubleRow
    from concourse.tile import TileContext as TC

    nc = bass.Bass(trn_type="TRN2")

    # ---- I/O -----------------------------------------------------------
    x_d = nc.dram_tensor("x", [C, HALF], fp32, kind="ExternalInput")
    xbf_d = nc.dram_tensor("x_bf", [C, HW], bf16, kind="ExternalInput")
    wq_d = nc.dram_tensor("wq_t", [C, C], bf16, kind="ExternalInput")
    wk_d = nc.dram_tensor("wk_t", [C, C], bf16, kind="ExternalInput")
    wv_d = nc.dram_tensor("wv_t", [C, C], bf16, kind="ExternalInput")
    wp_d = nc.dram_tensor("wp_t", [C, C], bf16, kind="ExternalInput")
    bq_d = nc.dram_tensor("bq", [C, 1], fp32, kind="ExternalInput")
    bv_d = nc.dram_tensor("bv", [C, 1], fp32, kind="ExternalInput")
    bp_d = nc.dram_tensor("bp", [C, 1], fp32, kind="ExternalInput")
    gnb_d = nc.dram_tensor("gn_b", [C, 1], fp32, kind="ExternalInput")
    gmat_d = nc.dram_tensor("gmat", [C, GROUPS], fp32, kind="ExternalInput")
    gbc_d = nc.dram_tensor("gbc", [GROUPS, C], fp32, kind="ExternalInput")
    ones2_d = nc.dram_tensor("ones2", [C, 2, 32], fp8, kind="ExternalInput")
    oner_d = nc.dram_tensor("ones_row", [1, C], mybir.dt.float32r, kind="ExternalInput")
    out_d = nc.dram_tensor("out", [C, HALF], fp32, kind="ExternalOutput")

    with TC(nc) as tc, tc.tile_pool(name="main", bufs=1) as pool, tc.tile_pool(
        name="psum", bufs=1, space="PSUM"
    ) as psum:
        # ---- ACT table prewarm (hide the exp table load) ---------------
        dum = pool.tile([1, 2], fp32, name="dum")
        nc.vector.memset(dum[:], 0.0)

        # ---- SBUF tiles -------------------------------------------------
        x_bf = pool.tile([C, HW], bf16, name="x_bf")
        x_sb = pool.tile([C, HALF], fp32, name="x_sb")
        wq_sb = pool.tile([C, C], bf16, name="wq_sb")
        wk_sb = pool.tile([C, C], bf16, name="wk_sb")
        wv_sb = pool.tile([C, C], bf16, name="wv_sb")
        wp_sb = pool.tile([C, C], bf16, name="wp_sb")
        wqs_sb = pool.tile([C, C], bf16, name="wqs_sb")
        wks_sb = pool.tile([C, C], bf16, name="wks_sb")
        wvs_sb = pool.tile([C, C], bf16, name="wvs_sb")
        bq_sb = pool.tile([C, 1], fp32, name="bq_sb")
        oner_sb = pool.tile([1, C], mybir.dt.float32r, name="oner_sb")
        bv_sb = pool.tile([C, 1], fp32, name="bv_sb")
        bp_sb = pool.tile([C, 1], fp32, name="bp_sb")
        gnb_sb = pool.tile([C, 1], fp32, name="gnb_sb")
        gmat_sb = pool.tile([C, GROUPS], fp32, name="gmat_sb")
        gbc_sb = pool.tile([GROUPS, C], fp32, name="gbc_sb")
        ones2_sb = pool.tile([C, 2, 32], fp8, name="ones2_sb")

        # ---- DMAs -------------------------------------------------------
        # SP queue: x_bf chunks 0-3 (stats-critical), then weights/consts
        # in order of first use; ACT queue: x_bf 4-5; Pool queue: x_bf
        # 6-7, then the late consts + residual x.
        for c4 in range(2):
            sl = slice(1024 * c4, 1024 * (c4 + 1))
            nc.sync.dma_start(x_bf[:, sl], xbf_d[:, sl])
        nc.scalar.dma_start(x_bf[:, 2048:3072], xbf_d[:, 2048:3072])
        nc.gpsimd.dma_start(x_bf[:, 3072:4096], xbf_d[:, 3072:4096])
        nc.sync.dma_start(wq_sb[:], wq_d[:])
        nc.sync.dma_start(wk_sb[:], wk_d[:])
        nc.sync.dma_start(gmat_sb[:], gmat_d[:])
        nc.sync.dma_start(gbc_sb[:], gbc_d[:])
        nc.sync.dma_start(gnb_sb[:], gnb_d[:])
        nc.sync.dma_start(bq_sb[:], bq_d[:])
        nc.sync.dma_start(wv_sb[:], wv_d[:])
        nc.sync.dma_start(wp_sb[:], wp_d[:])
        # prewarm exp/ln/square table while the stats DMAs stream
        nc.scalar.activation(dum[:], dum[:], AF.Exp)
        nc.sync.dma_start(oner_sb[:], oner_d[:])
        nc.sync.dma_start(bv_sb[:], bv_d[:])
        nc.sync.dma_start(bp_sb[:], bp_d[:])
        nc.sync.dma_start(ones2_sb[:], ones2_d[:])
        for c4 in range(4):
            sl = slice(512 * c4, 512 * (c4 + 1))
            nc.sync.dma_start(x_sb[:, sl], x_d[:, sl])

        # ---- GroupNorm stats (overlap the x DMA) -----------------------
        # Sums: pairwise-add tree on Pool (otherwise idle; fp32 partials).
        # Sum of squares: split between ACT (Square + accumulate over the
        # first half; shares the exp table set) and DVE (square + reduce
        # of the second half).
        sq_scr = pool.tile([C, 2048], bf16, name="sq_scr")
        sqd_scr = pool.tile([C, 2048], bf16, name="sqd_scr")
        ssb = pool.tile([C, 2], fp32, name="ssb")
        stats = pool.tile([C, 2], fp32, name="stats")
        tv = pool.tile([C, 1024], fp32, name="tv")
        tu = pool.tile([C, 1024], fp32, name="tu")
        tw = pool.tile([C, 1024], fp32, name="tw")
        nc.gpsimd.tensor_tensor(
            tv[:], x_bf[:, 2048:3072], x_bf[:, 3072:4096], ALU.add
        )
        nc.gpsimd.tensor_tensor(tu[:], x_bf[:, 0:1024], x_bf[:, 1024:2048], ALU.add)
        nc.gpsimd.tensor_tensor(tw[:], tu[:], tv[:], ALU.add)
        lvl = tw
        for width in (512, 256, 128, 64, 32):
            nxt = pool.tile([C, width], fp32, name=f"tl{width}")
            nc.gpsimd.tensor_tensor(
                nxt[:], lvl[:, :width], lvl[:, width : 2 * width], ALU.add
            )
            lvl = nxt
        nc.vector.tensor_reduce(stats[:, 0:1], lvl[:], axis=AX.X, op=ALU.add)
        nc.vector.tensor_tensor(
            sqd_scr[:], x_bf[:, 2048:4096], x_bf[:, 2048:4096], ALU.mult
        )
        nc.scalar.activation(
            sq_scr[:], x_bf[:, 0:2048], AF.Square, accum_out=ssb[:, 0:1]
        )
        nc.vector.tensor_reduce(ssb[:, 1:2], sqd_scr[:], axis=AX.X, op=ALU.add)
        nc.vector.tensor_reduce(stats[:, 1:2], ssb[:], axis=AX.X, op=ALU.add)

        eps_sb = pool.tile([GROUPS, 1], fp32, name="eps_sb")
        nc.vector.memset(eps_sb[:], EPS)

        gsum_ps = psum.tile([GROUPS, 2], fp32, name="gsum_ps", tag="u", bufs=2)
        nc.tensor.matmul(gsum_ps[:], gmat_sb[:], stats[:], start=True, stop=True)
        me2 = pool.tile([GROUPS, 2], fp32, name="me2")
        nc.vector.tensor_copy(me2[:], gsum_ps[:])

        msq = pool.tile([GROUPS, 1], fp32, name="msq")
        nc.vector.tensor_tensor(msq[:], me2[:, 0:1], me2[:, 0:1], ALU.mult)
        tve = pool.tile([GROUPS, 1], fp32, name="tve")
        nc.vector.tensor_tensor(tve[:], me2[:, 1:2], msq[:], ALU.subtract)

        # rsqrt(var+eps) = exp(-0.5*ln(var+eps)); eps rides the Ln bias.
        lnt = pool.tile([GROUPS, 1], fp32, name="lnt")
        nc.scalar.activation(lnt[:], tve[:], AF.Ln, bias=eps_sb[:])
        r1 = pool.tile([GROUPS, 1], fp32, name="r1")
        nc.scalar.activation(r1[:], lnt[:], AF.Exp, scale=-0.5)
        mr = pool.tile([GROUPS, 1], fp32, name="mr")
        nc.vector.tensor_tensor(mr[:], me2[:, 0:1], r1[:], ALU.mult)

        # a = gn_w * rsqrt (per channel), b = gn_b - mean * a
        a_ps = psum.tile([C, 1], fp32, name="a_ps", tag="u", bufs=2)
        nc.tensor.matmul(a_ps[:], gbc_sb[:], r1[:], start=True, stop=True)
        bm_ps = psum.tile([C, 1], fp32, name="bm_ps", tag="u", bufs=2)
        nc.tensor.matmul(bm_ps[:], gbc_sb[:], mr[:], start=True, stop=True)
        # fold the GN scale into the conv weights: w'[c,o] = w_t[c,o]*a[c]
        # (the per-partition scalar reads PSUM directly: walrus's one-PSUM
        # rule only covers non-scalar inputs)
        nc.vector.tensor_scalar(wqs_sb[:], wq_sb[:], a_ps[:], None, ALU.mult)
        nc.vector.tensor_scalar(wks_sb[:], wk_sb[:], a_ps[:], None, ALU.mult)
        nc.vector.tensor_scalar(wvs_sb[:], wv_sb[:], a_ps[:], None, ALU.mult)
        b_bf = pool.tile([C, 1], bf16, name="b_bf")
        nc.vector.tensor_tensor(b_bf[:], gnb_sb[:], bm_ps[:], ALU.subtract)
        # q-conv bias column (emitted before the convs so the evacs can
        # apply it as a per-partition scalar with no ordering hazard)
        bhq_ps = psum.tile([C, 1], fp32, name="bhq_ps", tag="u", bufs=2)
        nc.tensor.matmul(bhq_ps[:], wq_sb[:], b_bf[:], start=True, stop=True)
        bhq_sb = pool.tile([C, 1], fp32, name="bhq_sb")
        nc.vector.tensor_tensor(bhq_sb[:], bhq_ps[:], bq_sb[:], ALU.add)

        def emit_biases():
            """v/p conv biases from the GN shift b (emitted after the first
            q/k conv matmuls so PE serves those first):
              bhv = wv.b + bv ; bp2 = wp.bhv + bp
            (the k-conv bias is constant per query in the logits -> cancels)
            """
            bhv_ps = psum.tile([C, 1], fp32, name="bhv_ps", tag="u", bufs=2)
            nc.tensor.matmul(bhv_ps[:], wv_sb[:], b_bf[:], start=True, stop=True)
            bhv_sb = pool.tile([C, 1], fp32, name="bhv_sb")
            nc.vector.tensor_tensor(bhv_sb[:], bhv_ps[:], bv_sb[:], ALU.add)
            bhv_bf = pool.tile([C, 1], bf16, name="bhv_bf")
            nc.vector.tensor_copy(bhv_bf[:], bhv_sb[:])
            pb_ps = psum.tile([C, 1], fp32, name="pb_ps", tag="u", bufs=2)
            nc.tensor.matmul(pb_ps[:], wp_sb[:], bhv_bf[:], start=True, stop=True)
            bp2_sb = pool.tile([C, 1], fp32, name="bp2_sb")
            nc.vector.tensor_tensor(bp2_sb[:], pb_ps[:], bp_sb[:], ALU.add)
            # xb = x + bp2 (residual + folded projection bias), on Pool
            nc.gpsimd.tensor_scalar(xb[:], x_sb[:], bp2_sb[:], None, ALU.add)

        xb = pool.tile([C, HALF], fp32, name="xb")

        # ---- conv emitters ---------------------------------------------
        k_bf = pool.tile([C, HW], bf16, name="k_bf")
        q_bf = pool.tile([C, HALF], bf16, name="q_bf")
        vT_f8 = pool.tile([C, 32, C], fp8, name="vT_f8")

        def emit_k_chunk(c8):
            sl = slice(512 * c8, 512 * (c8 + 1))
            kps = psum.tile([C, 512], fp32, name=f"kps{c8}", tag="u", bufs=2)
            nc.tensor.matmul(kps[:], wks_sb[:], x_bf[:, sl], start=True, stop=True)
            if c8 == 0:
                # split so the first S^T matmuls start per key block
                nc.vector.tensor_copy(k_bf[:, 0:128], kps[:, 0:128])
                nc.vector.tensor_copy(k_bf[:, 128:384], kps[:, 128:384])
                nc.vector.tensor_copy(k_bf[:, 384:512], kps[:, 384:512])
            else:
                nc.vector.tensor_copy(k_bf[:, sl], kps[:])

        def emit_q_chunk(c4):
            sl = slice(512 * c4, 512 * (c4 + 1))
            qps = psum.tile([C, 512], fp32, name=f"qps{c4}", tag="u", bufs=2)
            nc.tensor.matmul(qps[:], wqs_sb[:], x_bf[:, sl], start=True, stop=True)
            if c4 == 0:
                # ACT is idle pre-body: overlap the q evac with the k evac
                nc.scalar.activation(q_bf[:, sl], qps[:], AF.Identity, bias=bhq_sb[:])
            else:
                nc.vector.tensor_scalar(q_bf[:, sl], qps[:], bhq_sb[:], None, ALU.add)

        def emit_v_chunk(g8):
            vps = psum.tile([C, 512], fp32, name=f"vps{g8}", tag="u", bufs=2)
            for m in range(4):
                jb = 4 * g8 + m
                nc.tensor.matmul(
                    vps[:, 128 * m : 128 * (m + 1)],
                    x_bf[:, 128 * jb : 128 * (jb + 1)],
                    wvs_sb[:],
                    start=True,
                    stop=True,
                )
            nc.vector.tensor_copy(
                vT_f8[:, 4 * g8 : 4 * (g8 + 1), :],
                vps[:].rearrange("p (m c) -> p m c", m=4),
            )

        emit_q_chunk(0)
        emit_k_chunk(0)
        emit_biases()

        # ---- attention --------------------------------------------------
        jgroups = [(3 * g, 3) for g in range(10)] + [(30, 2)]
        n_ib = HALF // 512  # 4 query blocks of 512
        pT_tiles = [None] * n_ib
        yps_tiles = [None] * n_ib
        den_tiles = [None] * n_ib
        out_sb = pool.tile([C, HALF], fp32, name="out_sb")

        conv_state = {"k": 1, "q": 1, "v": 0}
        # conv chunk emission schedule for block 0 (group -> jobs).
        blk0_jobs = {
            0: ["k", "v"], 1: ["k", "v"], 2: ["k", "v"], 3: ["k", "q", "v"],
            4: ["k", "v"], 5: ["k", "q", "v"], 6: ["k", "v"], 7: ["k", "q", "v"],
        }

        def run_conv_job(j):
            if j == "k" and conv_state["k"] < 8:
                emit_k_chunk(conv_state["k"])
                conv_state["k"] += 1
            elif j == "q" and conv_state["q"] < 4:
                emit_q_chunk(conv_state["q"])
                conv_state["q"] += 1
            elif j == "v" and conv_state["v"] < 8:
                emit_v_chunk(conv_state["v"])
                conv_state["v"] += 1

        def alloc_y(ib):
            yps_tiles[ib] = psum.tile([C, 512], fp32, name=f"yps{ib}", tag="u", bufs=2)

        def alloc_d(ib):
            # 32 identical rows: dual-fp8 ldweights needs >=32 weight cols
            den_tiles[ib] = psum.tile([32, 512], fp32, name=f"den{ib}", tag="u", bufs=2)

        def emit_av_pair(ib, p):
            nc.tensor.matmul(
                yps_tiles[ib][:],
                vT_f8[:, 2 * p : 2 * p + 2, :],
                pT_tiles[ib][:, 2 * p : 2 * p + 2, :],
                start=(p == 0),
                stop=(p == 15),
                perf_mode=DR,
            )

        def emit_den_pair(ib, p):
            nc.tensor.matmul(
                den_tiles[ib][:],
                ones2_sb[:],
                pT_tiles[ib][:, 2 * p : 2 * p + 2, :],
                start=(p == 0),
                stop=(p == 15),
                perf_mode=DR,
            )

        rbc_tiles = [None] * n_ib
        ybf_tiles = [None] * n_ib

        def emit_epi_a(ib):
            """1/den (fp32r) + y evacuation + 1/den row-broadcast matmul;
            emitted at the end of the block that ran AV(ib)/den(ib)."""
            rden = pool.tile(
                [1, 512], mybir.dt.float32r, name=f"rden{ib}", tag="rden", bufs=2
            )
            with nc.allow_low_precision(reason="fp32r broadcast of 1/den"):
                nc.vector.reciprocal(rden[:], den_tiles[ib][0:1, :])
            y_bf = pool.tile([C, 512], bf16, name=f"ybf{ib}", tag="ybf", bufs=2)
            if ib == 3:
                # ACT is idle after the last exp: do the y evac there
                nc.scalar.activation(y_bf[:], yps_tiles[ib][:], AF.Copy)
            else:
                nc.vector.tensor_copy(y_bf[:], yps_tiles[ib][:])
            ybf_tiles[ib] = y_bf
            rbc = psum.tile([C, 512], fp32, name=f"rbc{ib}", tag="u", bufs=2)
            nc.tensor.matmul(rbc[:], oner_sb[:], rden[:], start=True, stop=True)
            rbc_tiles[ib] = rbc

        def emit_epi_b(ib):
            """y/den -> proj -> + (x + bp2) -> store."""
            sl = slice(512 * ib, 512 * (ib + 1))
            if ib == 3:
                # fully halved pipeline to shorten the final drain
                y2 = pool.tile([C, 512], bf16, name=f"y2_{ib}", tag="y2", bufs=2)
                for hs in (slice(0, 256), slice(256, 512)):
                    nc.vector.tensor_tensor(
                        y2[:, hs],
                        ybf_tiles[ib][:, hs],
                        rbc_tiles[ib][:, hs],
                        ALU.mult,
                    )
                pps_h = []
                for hi, hs in enumerate((slice(0, 256), slice(256, 512))):
                    pps = psum.tile(
                        [C, 256], fp32, name=f"pps{ib}{hi}", tag="u", bufs=2
                    )
                    nc.tensor.matmul(
                        pps[:], wp_sb[:], y2[:, hs], start=True, stop=True
                    )
                    pps_h.append(pps)
                for hi, hs in enumerate((slice(0, 256), slice(256, 512))):
                    osl = slice(512 * ib + hs.start, 512 * ib + hs.stop)
                    nc.vector.tensor_tensor(
                        out_sb[:, osl], pps_h[hi][:], xb[:, osl], ALU.add
                    )
                    # the two final stores drain on separate DMA queues
                    eng = nc.sync if hi == 0 else nc.scalar
                    eng.dma_start(out_d[:, osl], out_sb[:, osl])
            else:
                y2 = pool.tile([C, 512], bf16, name=f"y2_{ib}", tag="y2", bufs=2)
                nc.vector.tensor_tensor(
                    y2[:], ybf_tiles[ib][:], rbc_tiles[ib][:], ALU.mult
                )
                pps = psum.tile([C, 512], fp32, name=f"pps{ib}", tag="u", bufs=2)
                nc.tensor.matmul(pps[:], wp_sb[:], y2[:], start=True, stop=True)
                nc.vector.tensor_tensor(out_sb[:, sl], pps[:], xb[:, sl], ALU.add)
                nc.sync.dma_start(out_d[:, sl], out_sb[:, sl])

        def emit_st_group(ib, j0, glen):
            st = psum.tile([C, glen, 512], fp32, name=f"st{ib}_{j0}", tag="st", bufs=2)
            qs = q_bf[:, 512 * ib : 512 * (ib + 1)]
            for u2 in range(glen):
                jb = j0 + u2
                nc.tensor.matmul(
                    st[:, u2, :],
                    k_bf[:, 128 * jb : 128 * (jb + 1)],
                    qs,
                    start=True,
                    stop=True,
                )
            nc.scalar.activation(
                pT_tiles[ib][:, j0 : j0 + glen, :], st[:], AF.Exp, scale=float(SCL)
            )

        # -- block 0: S^T/exp + remaining conv chunks ---------------------
        pT_tiles[0] = pool.tile([C, 32, 512], fp8, name="pT0", tag="pT", bufs=3)
        for gi, (j0, glen) in enumerate(jgroups):
            emit_st_group(0, j0, glen)
            for j in blk0_jobs.get(gi, []):
                run_conv_job(j)

        # -- block 1: + AV0/den0 spread over the block --------------------
        pT_tiles[1] = pool.tile([C, 32, 512], fp8, name="pT1", tag="pT", bufs=3)
        av_done = den_done = 0
        for gi, (j0, glen) in enumerate(jgroups):
            emit_st_group(1, j0, glen)
            if gi == 1:
                alloc_d(0)
                alloc_y(0)
            if gi >= 1:
                tgt = min(16, 2 * gi)
                while den_done < tgt:
                    emit_den_pair(0, den_done)
                    den_done += 1
                while av_done < tgt:
                    emit_av_pair(0, av_done)
                    av_done += 1
        while den_done < 16:
            emit_den_pair(0, den_done)
            den_done += 1
        while av_done < 16:
            emit_av_pair(0, av_done)
            av_done += 1
        emit_epi_a(0)

        # -- block 2: + epilogue-B(0), AV1/den1 spread --------------------
        pT_tiles[2] = pool.tile([C, 32, 512], fp8, name="pT2", tag="pT", bufs=3)
        av_done = den_done = 0
        for gi, (j0, glen) in enumerate(jgroups):
            emit_st_group(2, j0, glen)
            if gi == 0:
                emit_epi_b(0)   # praw0/tmul0 free the rbc0/pps0 slots
            if gi == 2:
                alloc_d(1)      # A slot: after tmul0 read of rbc0
                alloc_y(1)      # B slot: after praw0 read of pps0
            if gi >= 2:
                tgt = min(16, 2 * (gi - 1))
                while den_done < tgt:
                    emit_den_pair(1, den_done)
                    den_done += 1
                while av_done < tgt:
                    emit_av_pair(1, av_done)
                    av_done += 1
        while den_done < 16:
            emit_den_pair(1, den_done)
            den_done += 1
        while av_done < 16:
            emit_av_pair(1, av_done)
            av_done += 1
        emit_epi_a(1)

        # -- block 3: epi-B(1), AV2/den2 bursts, self-trailing AV3/den3 --
        pT_tiles[3] = pool.tile([C, 32, 512], fp8, name="pT3", tag="pT", bufs=3)
        av2 = den2 = 0
        av3 = den3 = 0
        for gi, (j0, glen) in enumerate(jgroups):
            emit_st_group(3, j0, glen)
            if gi == 0:
                emit_epi_b(1)
            if gi == 2:
                alloc_d(2)
                alloc_y(2)
            if 2 <= gi <= 5:
                tgt = min(16, 4 * (gi - 1))
                while den2 < tgt:
                    emit_den_pair(2, den2)
                    den2 += 1
                while av2 < tgt:
                    emit_av_pair(2, av2)
                    av2 += 1
            if gi == 5:
                emit_epi_a(2)   # recip2/y-evac2 free den2/yps2 mid-block
            if gi == 6:
                emit_epi_b(2)
            if gi >= 7:
                if den3 == 0:
                    alloc_d(3)  # A slot: after tmul2 read of rbc2
                    alloc_y(3)  # B slot: after praw2 read of pps2
                ready = min(16, (3 * gi + 1) // 2 + 1)
                while den3 < ready:
                    emit_den_pair(3, den3)
                    den3 += 1
                while av3 < ready:
                    emit_av_pair(3, av3)
                    av3 += 1
        while den3 < 16:
            emit_den_pair(3, den3)
            den3 += 1
        while av3 < 16:
            emit_av_pair(3, av3)
            av3 += 1
        emit_epi_a(3)
        emit_epi_b(3)

    _split_excess_waits(nc)
    return nc


def _get_nc():
    if "nc" not in _CACHE:
        _CACHE["nc"] = _build_bass()
    return _CACHE["nc"]


def prepare_in_maps(x, gn_w, gn_b, wq, bq, wk, bk, wv, bv, wp, bp):
    import ml_dtypes

    bf = ml_dtypes.bfloat16
    f8 = ml_dtypes.float8_e4m3
    f32 = np.float32

    x = np.asarray(x, f32)
    xf = x.reshape(B, C, HW)

    def col(v):
        return np.ascontiguousarray(np.asarray(v, f32).reshape(C, 1))

    wq_t = np.ascontiguousarray(np.asarray(wq, f32).T).astype(bf)
    wk_t = np.ascontiguousarray(np.asarray(wk, f32).T).astype(bf)
    wv_t = np.ascontiguousarray(np.asarray(wv, f32).T).astype(bf)
    wp_t = np.ascontiguousarray(np.asarray(wp, f32).T).astype(bf)

    gmat = np.zeros((C, GROUPS), f32)
    for c in range(C):
        gmat[c, c // GSIZE] = 1.0
    gbc = np.ascontiguousarray(gmat.T * np.asarray(gn_w, f32)[None, :])
    gmat = gmat * f32(1.0 / NPIX_G)

    shared = {
        "wq_t": wq_t,
        "wk_t": wk_t,
        "wv_t": wv_t,
        "wp_t": wp_t,
        "bq": col(bq),
        "bv": col(bv),
        "bp": col(bp),
        "gn_b": col(gn_b),
        "gmat": gmat,
        "gbc": gbc,
        "ones2": np.ones((C, 2, 32), f8),
        "ones_row": np.ones((1, C), f32),
    }

    in_maps = []
    for core in range(NCORES):
        b, qh = divmod(core, 2)
        if qh == 0:
            xp = np.ascontiguousarray(xf[b])
        else:
            xp = np.ascontiguousarray(
                np.concatenate([xf[b][:, HALF:], xf[b][:, :HALF]], axis=1)
            )
        in_maps.append(
            {
                "x": np.ascontiguousarray(xp[:, :HALF]),
                "x_bf": xp.astype(bf),
                **shared,
            }
        )
    return in_maps


def kernel(x, gn_w, gn_b, wq, bq, wk, bk, wv, bv, wp, bp):
    from concourse.bass_utils import run_bass_kernel_spmd

    f32 = np.float32
    in_maps = prepare_in_maps(x, gn_w, gn_b, wq, bq, wk, bk, wv, bv, wp, bp)
    nc = _get_nc()
    res = run_bass_kernel_spmd(nc, in_maps, core_ids=list(range(NCORES)))

    out = np.empty((B, C, HW), f32)
    for core in range(NCORES):
        b, qh = divmod(core, 2)
        out[b][:, HALF * qh : HALF * (qh + 1)] = res.results[core]["out"]
    return out.reshape(B, C, H, W)
